# revision 1
# baseline (speedup 1.0000x reference)
"""BiAttention similarity kernel for Trainium2, 8-core data-parallel over batch.

Computes, per batch b:
    s0 = c @ c_weight                  # [L, 1]
    s1 = (c @ q_weight)^T              # [1, L]
    s2 = (c * cq_weight) @ q^T         # [L, L]
    s  = s0 + s1 + s2 + bias           # [L, L]

Shapes (hardcoded): B=8, L=2048, D=256, fp32 in/out.

Distribution strategy: data-parallel over batch, one batch per core. The
host-side sharding step hands each core its shard in the layout the PE
array consumes: d-major (transposed) fp16. All arithmetic — cq_weight
scaling, s0/s1 reductions, the GEMM, and the broadcast adds — runs on
device:
  - q^T scaled by cq_weight per-partition (d on partitions after transpose)
  - s0/s1 rows via skinny matmuls against c^T
  - main tiles: one PSUM accumulation group of 3 matmuls per [128, 512] tile
    (weight-stationary: each lhsT held across the 4 column tiles of a row chunk):
      K=2  : [s0_row; ones]^T @ [ones; s1_row + bias]   (adds s0[i] + s1[j] + bias)
      K=128: cT0^T @ qmodT0
      K=128: cT1^T @ qmodT1
  - PSUM->SBUF copy split between ScalarE and VectorE
  - 1 MiB contiguous output DMAs
"""

import numpy as np
from contextlib import ExitStack

import concourse.bass as bass
import concourse.tile as tile
from concourse import bacc, mybir
from concourse.bass_utils import run_bass_kernel_spmd

F32 = mybir.dt.float32
F16 = mybir.dt.float16

B = 8
L = 2048
D = 256
NK = D // 128          # 2 contraction chunks of 128
NI = L // 128          # 16 row chunks
MAIN_N = 512           # moving free dim; matmul output must stay in one PSUM bank
NJ = L // MAIN_N

# set by test harness to request an NTFF trace; results stashed in LAST_RESULTS
TRACE = False
LAST_RESULTS = None

_NC_CACHE = None


def build_body(ctx: ExitStack, tc: tile.TileContext, aps: dict):
    nc = tc.nc
    ct_d, qt_d, cw_d, qw_d, cqw_d, bias_d, s_d = (
        aps["ct"], aps["qt"], aps["c_weight"], aps["q_weight"],
        aps["cq_weight"], aps["bias"], aps["s"],
    )

    consts = ctx.enter_context(tc.tile_pool(name="consts", bufs=1))
    psum = ctx.enter_context(tc.tile_pool(name="psum", bufs=8, space="PSUM"))
    outp = ctx.enter_context(tc.tile_pool(name="outp", bufs=16))

    # ---- small constants -------------------------------------------------
    cw16 = consts.tile([128, NK], F16)
    nc.gpsimd.dma_start(cw16[:], cw_d.rearrange("(k p) one -> p (k one)", p=128))
    qw16 = consts.tile([128, NK], F16)
    nc.gpsimd.dma_start(qw16[:], qw_d.rearrange("(k p) one -> p (k one)", p=128))
    cqw32 = consts.tile([128, NK], F32)
    nc.gpsimd.dma_start(cqw32[:], cqw_d.rearrange("a b (k p) -> p (a b k)", p=128))
    bias_sb = consts.tile([1, 1], F32)
    nc.gpsimd.dma_start(bias_sb[:], bias_d[None, :])

    # transposed fp16 operands: cT_k[d, i], qmodT_k[d, j] for d-chunk k,
    # loaded in column quarters for finer-grained downstream readiness.
    cT = [consts.tile([128, L], F16, tag=f"cT{k}", name=f"cT{k}")
          for k in range(NK)]
    qT = [consts.tile([128, L], F16, tag=f"qT{k}", name=f"qT{k}")
          for k in range(NK)]
    # c^T quarters on the SP HWDGE ring (rows phase consumes c^T first and
    # gates everything downstream), q^T halves on the ACT HWDGE ring.
    for quad in range(4):
        qsl = slice(quad * 512, (quad + 1) * 512)
        for k in range(NK):
            ksl = slice(k * 128, (k + 1) * 128)
            nc.sync.dma_start(cT[k][:, qsl], ct_d[ksl, qsl])
    for half in range(2):
        hsl = slice(half * 1024, (half + 1) * 1024)
        for k in range(NK):
            ksl = slice(k * 128, (k + 1) * 128)
            nc.scalar.dma_start(qT[k][:, hsl], qt_d[ksl, hsl])
            # qmodT = qT * cq_weight (per-partition scalar after transpose)
            nc.vector.tensor_scalar_mul(qT[k][:, hsl], qT[k][:, hsl],
                                        cqw32[:, k:k + 1])

    # augmented-K rows
    ex_lhs = consts.tile([2, L], F16)   # p0 = s0 row, p1 = ones
    ex_rhs = consts.tile([2, L], F16)   # p0 = ones,   p1 = s1 row + bias
    s1_stage = consts.tile([1, L], F16)
    nc.gpsimd.memset(ex_lhs[0:2, :], 1.0)   # p0 overwritten by s0 row below
    nc.gpsimd.memset(ex_rhs[0:2, :], 1.0)   # p1 overwritten by s1 row below

    # ---- s0 / s1 rows ----------------------------------------------------
    # s0 = c @ c_weight, s1 = c @ q_weight; both as [1, L] rows via
    # out[1, N] = w_chunk[128, 1]^T @ cT_chunk[128, N], accumulated over k.
    for jj in range(4):
        sl = slice(jj * 512, (jj + 1) * 512)
        row0_ps = psum.tile([128, 512], F32, tag="main", name="row0_ps")
        row1_ps = psum.tile([128, 512], F32, tag="main", name="row1_ps")
        for k in range(NK):
            nc.tensor.matmul(row0_ps[0:1, :], cw16[:, k:k + 1], cT[k][:, sl],
                             start=(k == 0), stop=(k == NK - 1))
        for k in range(NK):
            nc.tensor.matmul(row1_ps[0:1, :], qw16[:, k:k + 1], cT[k][:, sl],
                             start=(k == 0), stop=(k == NK - 1))
        # s0 -> ex_lhs partition 0 (fp16 downcast on copy)
        nc.vector.tensor_copy(ex_lhs[0:1, sl], row0_ps[0:1, :])
        # s1 + bias -> staging row (partition 0), bounced to ex_rhs p1 by DMA
        nc.vector.tensor_scalar_add(s1_stage[0:1, sl], row1_ps[0:1, :],
                                    bias_sb[0:1, 0:1])
        nc.scalar.dma_start(ex_rhs[1:2, sl], s1_stage[0:1, sl])

    # ---- main loop: 16 row-chunks x (L/MAIN_N) column tiles --------------
    Copy = mybir.ActivationFunctionType.Copy
    for i in range(NI):
        isl = slice(i * 128, (i + 1) * 128)
        out_sb = outp.tile([128, L], F32, tag="out", name="out_sb")
        # weight-stationary: hold each lhsT across all NJ column tiles so its
        # LDWEIGHTS is paid once per sweep instead of once per matmul
        pss = [psum.tile([128, MAIN_N], F32, tag="main", name=f"ps{jj}")
               for jj in range(NJ)]
        for jj in range(NJ):
            nc.tensor.matmul(pss[jj][:], ex_lhs[:, isl],
                             ex_rhs[:, jj * MAIN_N:(jj + 1) * MAIN_N],
                             start=True, stop=False)
        for jj in range(NJ):
            nc.tensor.matmul(pss[jj][:], cT[0][:, isl],
                             qT[0][:, jj * MAIN_N:(jj + 1) * MAIN_N],
                             start=False, stop=False)
        for jj in range(NJ):
            nc.tensor.matmul(pss[jj][:], cT[1][:, isl],
                             qT[1][:, jj * MAIN_N:(jj + 1) * MAIN_N],
                             start=False, stop=True)
            # split the PSUM->SBUF copy between ScalarE and VectorE
            sl = slice(jj * MAIN_N, (jj + 1) * MAIN_N)
            if jj % 2 == 0:
                nc.scalar.activation(out_sb[:, sl], pss[jj][:], Copy)
            else:
                nc.vector.tensor_copy(out_sb[:, sl], pss[jj][:])
        # Sync issues both output halves (its waits are cheap; keeps ACT free)
        nc.sync.dma_start(s_d[isl, 0:1024], out_sb[:, 0:1024])
        nc.sync.dma_start(s_d[isl, 1024:2048], out_sb[:, 1024:2048])


def build_nc():
    nc = bacc.Bacc("TRN2", target_bir_lowering=False, debug=False)
    aps = {
        "ct": nc.dram_tensor("ct", [D, L], F16, kind="ExternalInput").ap(),
        "qt": nc.dram_tensor("qt", [D, L], F16, kind="ExternalInput").ap(),
        "c_weight": nc.dram_tensor("c_weight", [D, 1], F32,
                                   kind="ExternalInput").ap(),
        "q_weight": nc.dram_tensor("q_weight", [D, 1], F32,
                                   kind="ExternalInput").ap(),
        "cq_weight": nc.dram_tensor("cq_weight", [1, 1, D], F32,
                                    kind="ExternalInput").ap(),
        "bias": nc.dram_tensor("bias", [1], F32, kind="ExternalInput").ap(),
        "s": nc.dram_tensor("s", [L, L], F32, kind="ExternalOutput").ap(),
    }
    with tile.TileContext(nc) as tc:
        with ExitStack() as ctx:
            build_body(ctx, tc, aps)
    nc.compile()
    return nc


def get_nc():
    global _NC_CACHE
    if _NC_CACHE is None:
        _NC_CACHE = build_nc()
    return _NC_CACHE


def kernel(c, q, c_weight, q_weight, cq_weight, bias):
    global LAST_RESULTS
    nc = get_nc()
    c = np.asarray(c, dtype=np.float32)
    q = np.asarray(q, dtype=np.float32)
    cw = np.asarray(c_weight, dtype=np.float32)
    qw = np.asarray(q_weight, dtype=np.float32)
    cqw = np.asarray(cq_weight, dtype=np.float32)
    bias = np.asarray(bias, dtype=np.float32)
    # shard: batch b -> core b, shards laid out d-major (transposed) fp16
    in_maps = [
        {
            "ct": np.ascontiguousarray(c[b].T).astype(np.float16),
            "qt": np.ascontiguousarray(q[b].T).astype(np.float16),
            "c_weight": cw,
            "q_weight": qw,
            "cq_weight": cqw,
            "bias": bias,
        }
        for b in range(B)
    ]
    res = run_bass_kernel_spmd(nc, in_maps, core_ids=list(range(B)), trace=TRACE)
    LAST_RESULTS = res
    return np.stack([res.results[b]["s"] for b in range(B)], axis=0)



# revision 5
# speedup vs baseline: 1.2233x; 1.2233x over previous
"""BiAttention similarity kernel for Trainium2, 8-core data-parallel over batch.

Computes, per batch b:
    s0 = c @ c_weight                  # [L, 1]
    s1 = (c @ q_weight)^T              # [1, L]
    s2 = (c * cq_weight) @ q^T         # [L, L]
    s  = s0 + s1 + s2 + bias           # [L, L]

Shapes (hardcoded): B=8, L=2048, D=256, fp32 in/out (device math fp16/fp32,
device output fp16, upcast to fp32 on host).

Distribution strategy: data-parallel over batch, one batch per core. Host
hands each core its shard d-major (transposed) fp16; device output is fp16
(halves the dominant HBM write) and the host upcasts.

Device plan per core:
  - qmodT = qT * cq_weight (per-partition scalar after transpose), on DVE
  - S1B [128, L]: s1[j]+bias broadcast across partitions, computed directly
    on the PE as qw_bcast^T @ cT (+ ones^T @ bias_row), evacuated to SBUF
  - s0 column accumulated in a dedicated PSUM bank via N=1 matmuls that
    reuse the main loop's stationary cT chunks (no extra LDWEIGHTS)
  - main GEMM: 16 row chunks x 4 column tiles of [128, 512], K=256 as two
    K=128 passes accumulating in PSUM (pure data passes, no rank-1 padding)
  - PSUM evacuation fuses the rank-1 adds, split across three engines:
      D tiles: DVE scalar_tensor_tensor  out = (psum + s0col) + S1B
      C tiles: ScalarE activation (psum + s0col -> fp16 tmp), DVE add S1B
      G tiles: ScalarE activation (psum + s0col -> fp16 tmp), GpSimd add S1B
  - output DMAs: one 512 KiB transfer per row chunk on the Sync HWDGE ring
"""

import numpy as np
from contextlib import ExitStack

import concourse.bass as bass
import concourse.tile as tile
from concourse import bacc, mybir
from concourse.bass_utils import run_bass_kernel_spmd

F32 = mybir.dt.float32
F16 = mybir.dt.float16

B = 8
L = 2048
D = 256
NK = D // 128          # 2 contraction chunks of 128
NI = L // 128          # 16 row chunks
MAIN_N = 512           # matmul output must stay in one PSUM bank
NJ = L // MAIN_N       # 4 column tiles per row chunk

# set by test harness to request an NTFF trace; results stashed in LAST_RESULTS
TRACE = False
LAST_RESULTS = None

_NC_CACHE = None


def build_body(ctx: ExitStack, tc: tile.TileContext, aps: dict):
    nc = tc.nc
    ct_d, qt_d, cw_d, qw_d, cqw_d, bias_d, s_d = (
        aps["ct"], aps["qt"], aps["c_weight"], aps["q_weight"],
        aps["cq_weight"], aps["bias"], aps["s"],
    )
    Copy = mybir.ActivationFunctionType.Copy
    Ident = mybir.ActivationFunctionType.Identity
    ADD = mybir.AluOpType.add

    consts = ctx.enter_context(tc.tile_pool(name="consts", bufs=1))
    psum = ctx.enter_context(tc.tile_pool(name="psum", bufs=7, space="PSUM"))
    psum_s0 = ctx.enter_context(tc.tile_pool(name="psum_s0", bufs=1,
                                             space="PSUM"))
    outp = ctx.enter_context(tc.tile_pool(name="outp", bufs=3))
    tmpp = ctx.enter_context(tc.tile_pool(name="tmpp", bufs=6))

    # ---- small constants -------------------------------------------------
    cw16 = consts.tile([128, NK], F16)
    nc.gpsimd.dma_start(cw16[:], cw_d.rearrange("(k p) one -> p (k one)", p=128))
    qw32 = consts.tile([128, NK], F32)
    nc.gpsimd.dma_start(qw32[:], qw_d.rearrange("(k p) one -> p (k one)", p=128))
    cqw32 = consts.tile([128, NK], F32)
    nc.gpsimd.dma_start(cqw32[:], cqw_d.rearrange("a b (k p) -> p (a b k)", p=128))
    bias_sb = consts.tile([1, 1], F32)
    nc.gpsimd.dma_start(bias_sb[:], bias_d[None, :])

    ones_sb = consts.tile([128, MAIN_N], F16)
    nc.gpsimd.memset(ones_sb[:], 1.0)

    # ---- transposed fp16 operands ---------------------------------------
    # cT gates everything (stationary + S1B + s0) -> SP HWDGE ring, halves.
    # qT (moving operand) -> ACT HWDGE ring, halves, then scaled by cqw.
    cT = [consts.tile([128, L], F16, tag=f"cT{k}", name=f"cT{k}")
          for k in range(NK)]
    qT = [consts.tile([128, L], F16, tag=f"qT{k}", name=f"qT{k}")
          for k in range(NK)]
    for half in range(2):
        hsl = slice(half * 1024, (half + 1) * 1024)
        for k in range(NK):
            ksl = slice(k * 128, (k + 1) * 128)
            nc.sync.dma_start(cT[k][:, hsl], ct_d[ksl, hsl])
    for half in range(2):
        hsl = slice(half * 1024, (half + 1) * 1024)
        for k in range(NK):
            ksl = slice(k * 128, (k + 1) * 128)
            nc.scalar.dma_start(qT[k][:, hsl], qt_d[ksl, hsl])
            # qmodT = qT * cq_weight (per-partition scalar after transpose)
            nc.vector.tensor_scalar_mul(qT[k][:, hsl], qT[k][:, hsl],
                                        cqw32[:, k:k + 1])

    # ---- S1B: s1[j] + bias broadcast across all 128 partitions ----------
    # S1B[p, j] = sum_d qw[d] * cT[d, j] + bias, via qw_bcast lhsT whose
    # every column is qw (so each output partition gets the same row), plus
    # a K=1 ones^T @ bias_row pass for the bias.
    qw_bc = [consts.tile([128, 128], F16, tag=f"qwbc{k}", name=f"qwbc{k}")
             for k in range(NK)]
    for k in range(NK):
        nc.vector.tensor_scalar_mul(qw_bc[k][:], ones_sb[:, 0:128],
                                    qw32[:, k:k + 1])
    bias_row = consts.tile([1, MAIN_N], F16)
    nc.scalar.activation(bias_row[0:1, :], ones_sb[0:1, :], Copy,
                         scale=bias_sb[0:1, 0:1])

    s1b = consts.tile([128, L], F16, name="s1b")
    s1b_ps = [psum.tile([128, MAIN_N], F32, tag="main", name=f"s1b_ps{jj}")
              for jj in range(NJ)]
    for k in range(NK):
        for jj in range(NJ):
            jsl = slice(jj * MAIN_N, (jj + 1) * MAIN_N)
            nc.tensor.matmul(s1b_ps[jj][:], qw_bc[k][:], cT[k][:, jsl],
                             start=(k == 0), stop=False)
    for jj in range(NJ):
        nc.tensor.matmul(s1b_ps[jj][:], ones_sb[0:1, 0:128], bias_row[0:1, :],
                         start=False, stop=True)
    for jj in range(NJ):
        jsl = slice(jj * MAIN_N, (jj + 1) * MAIN_N)
        if jj % 2 == 0:
            nc.vector.tensor_copy(s1b[:, jsl], s1b_ps[jj][:])
        else:
            nc.scalar.activation(s1b[:, jsl], s1b_ps[jj][:], Copy)

    # ---- s0 column accumulator (one PSUM bank for all 16 chunks) --------
    s0acc = psum_s0.tile([128, NI], F32, name="s0acc")
    s0col = consts.tile([128, NI], F32, name="s0col")

    # ---- main loop: 16 row chunks x 4 column tiles -----------------------
    # Tile assignment per chunk: D = DVE fused, C = ACT+DVE, G = ACT+GpSimd
    assign_even = ("D", "C", "C", "G")
    assign_odd = ("D", "D", "C", "G")
    for i in range(NI):
        isl = slice(i * 128, (i + 1) * 128)
        out_sb = outp.tile([128, L], F16, tag="out", name="out_sb")
        pss = [psum.tile([128, MAIN_N], F32, tag="main", name=f"ps{jj}")
               for jj in range(NJ)]
        # weight-stationary: all matmuls of one k share lhsT = cT[k][:, isl],
        # including the N=1 s0 column matmul (rhs = c_weight chunk).
        for k in range(NK):
            for jj in range(NJ):
                nc.tensor.matmul(pss[jj][:], cT[k][:, isl],
                                 qT[k][:, jj * MAIN_N:(jj + 1) * MAIN_N],
                                 start=(k == 0), stop=(k == NK - 1))
            nc.tensor.matmul(s0acc[:, i:i + 1], cT[k][:, isl],
                             cw16[:, k:k + 1],
                             start=(k == 0), stop=(k == NK - 1))
        # bounce s0 column through SBUF (fp32) for the evacuation ops
        nc.scalar.activation(s0col[:, i:i + 1], s0acc[:, i:i + 1], Copy)

        assign = assign_even if i % 2 == 0 else assign_odd
        for jj in range(NJ):
            jsl = slice(jj * MAIN_N, (jj + 1) * MAIN_N)
            kind = assign[jj]
            if kind == "D":
                nc.vector.scalar_tensor_tensor(
                    out_sb[:, jsl], pss[jj][:], s0col[:, i:i + 1],
                    s1b[:, jsl], ADD, ADD)
            else:
                tmp = tmpp.tile([128, MAIN_N], F16, tag="tmp", name="tmp")
                nc.scalar.activation(tmp[:], pss[jj][:], Ident,
                                     bias=s0col[:, i:i + 1])
                if kind == "C":
                    nc.vector.tensor_tensor(out_sb[:, jsl], tmp[:],
                                            s1b[:, jsl], ADD)
                else:
                    nc.gpsimd.tensor_tensor(out_sb[:, jsl], tmp[:],
                                            s1b[:, jsl], ADD)
        nc.sync.dma_start(s_d[isl, :], out_sb[:])


def build_nc():
    nc = bacc.Bacc("TRN2", target_bir_lowering=False, debug=False)
    aps = {
        "ct": nc.dram_tensor("ct", [D, L], F16, kind="ExternalInput").ap(),
        "qt": nc.dram_tensor("qt", [D, L], F16, kind="ExternalInput").ap(),
        "c_weight": nc.dram_tensor("c_weight", [D, 1], F32,
                                   kind="ExternalInput").ap(),
        "q_weight": nc.dram_tensor("q_weight", [D, 1], F32,
                                   kind="ExternalInput").ap(),
        "cq_weight": nc.dram_tensor("cq_weight", [1, 1, D], F32,
                                    kind="ExternalInput").ap(),
        "bias": nc.dram_tensor("bias", [1], F32, kind="ExternalInput").ap(),
        "s": nc.dram_tensor("s", [L, L], F16, kind="ExternalOutput").ap(),
    }
    with tile.TileContext(nc) as tc:
        with ExitStack() as ctx:
            build_body(ctx, tc, aps)
    nc.compile()
    return nc


def get_nc():
    global _NC_CACHE
    if _NC_CACHE is None:
        _NC_CACHE = build_nc()
    return _NC_CACHE


def kernel(c, q, c_weight, q_weight, cq_weight, bias):
    global LAST_RESULTS
    nc = get_nc()
    c = np.asarray(c, dtype=np.float32)
    q = np.asarray(q, dtype=np.float32)
    cw = np.asarray(c_weight, dtype=np.float32)
    qw = np.asarray(q_weight, dtype=np.float32)
    cqw = np.asarray(cq_weight, dtype=np.float32)
    bias = np.asarray(bias, dtype=np.float32)
    # shard: batch b -> core b, shards laid out d-major (transposed) fp16
    in_maps = [
        {
            "ct": np.ascontiguousarray(c[b].T).astype(np.float16),
            "qt": np.ascontiguousarray(q[b].T).astype(np.float16),
            "c_weight": cw,
            "q_weight": qw,
            "cq_weight": cqw,
            "bias": bias,
        }
        for b in range(B)
    ]
    res = run_bass_kernel_spmd(nc, in_maps, core_ids=list(range(B)), trace=TRACE)
    LAST_RESULTS = res
    return np.stack([res.results[b]["s"].astype(np.float32) for b in range(B)],
                    axis=0)


# revision 6
# speedup vs baseline: 1.3663x; 1.1169x over previous
"""BiAttention similarity kernel for Trainium2, 8-core data-parallel over batch.

Computes, per batch b:
    s0 = c @ c_weight                  # [L, 1]
    s1 = (c @ q_weight)^T              # [1, L]
    s2 = (c * cq_weight) @ q^T         # [L, L]
    s  = s0 + s1 + s2 + bias           # [L, L]

Shapes (hardcoded): B=8, L=2048, D=256, fp32 in/out (device math fp16/fp32,
device output fp16, upcast to fp32 on host).

Distribution: data-parallel over batch, one batch per core. Host hands each
core its shard d-major (transposed) fp16 plus pre-packed per-partition weight
tiles; device output is fp16 (halves the dominant HBM write).

Device plan per core:
  - warmup matmuls on constant data during the input-load window so the PE's
    HAM clock gate reaches 2.4 GHz before real work arrives
  - S1B [128, L]: s1[j]+bias broadcast across partitions, computed on the PE
    as qw_bcast^T @ cT (+ ones^T @ bias_row), evacuated to SBUF fp16
  - s0 column accumulated in one PSUM bank via N=1 matmuls that reuse the
    main loop's stationary cT chunks, bounced [128,1] per chunk to SBUF
  - main GEMM: 16 row chunks; PSUM tiles are [128,1024] fp32 (two banks) so
    each evacuation instruction covers two banks' worth of output
  - evacuation fuses the rank-1 adds, split across three engines:
      a tiles: DVE scalar_tensor_tensor  out = (psum + s0col) + S1B
      b tiles: ScalarE activation (psum + s0col -> fp16 tmp), then
               DVE tensor_tensor (2x fp16 mode) or GpSimd adds S1B
  - output: one 512 KiB DMA per row chunk on the Sync HWDGE ring
"""

import numpy as np
from contextlib import ExitStack

import concourse.bass as bass
import concourse.tile as tile
from concourse import bacc, mybir
from concourse.bass_utils import run_bass_kernel_spmd

F32 = mybir.dt.float32
F16 = mybir.dt.float16

B = 8
L = 2048
D = 256
NK = D // 128          # 2 contraction chunks of 128
NI = L // 128          # 16 row chunks
MAIN_N = 512           # one matmul output <= one PSUM bank
BIG_N = 1024           # evacuation tile: two PSUM banks
N_WARM = 6             # PE warmup matmuls during input load
GPS_CHUNKS = {1, 3, 5, 7, 9, 11, 13, 14}   # chunks whose b-tile partner is GpSimd

TRACE = False
LAST_RESULTS = None

_NC_CACHE = None


def build_body(ctx: ExitStack, tc: tile.TileContext, aps: dict):
    nc = tc.nc
    ct_d, qt_d, w16_d, w32_d, s_d = (
        aps["ct"], aps["qt"], aps["w16"], aps["w32"], aps["s"],
    )
    Copy = mybir.ActivationFunctionType.Copy
    Ident = mybir.ActivationFunctionType.Identity
    ADD = mybir.AluOpType.add

    consts = ctx.enter_context(tc.tile_pool(name="consts", bufs=1))
    psum = ctx.enter_context(tc.tile_pool(name="psum", bufs=3, space="PSUM"))
    psum_s0 = ctx.enter_context(tc.tile_pool(name="psum_s0", bufs=1,
                                             space="PSUM"))
    outp = ctx.enter_context(tc.tile_pool(name="outp", bufs=3))
    tmpp = ctx.enter_context(tc.tile_pool(name="tmpp", bufs=3))

    # ---- packed constants: one fast HWDGE load each -----------------------
    # w16[p] = [cw[p], cw[128+p]] fp16; w32[p] = [cqw k0, cqw k1, qw k0,
    # qw k1, bias] fp32 (bias only meaningful on partition 0)
    w16 = consts.tile([128, NK], F16, name="w16")
    nc.sync.dma_start(w16[:], w16_d[:, :])
    w32 = consts.tile([128, 5], F32, name="w32")
    nc.sync.dma_start(w32[:], w32_d[:, :])
    cw16 = w16
    cqw32 = w32[:, 0:NK]
    qw32 = w32[:, NK:2 * NK]
    bias_sb = w32[0:1, 4:5]

    ones_sb = consts.tile([128, MAIN_N], F16, name="ones_sb")
    nc.vector.memset(ones_sb[:], 1.0)

    # ---- PE warmup: release the HAM clock gate during the load window ----
    # s0acc's bank doubles as the warmup target; junk lands in columns the
    # s0 matmuls never touch (and start=True clears has_written anyway).
    s0acc = psum_s0.tile([128, MAIN_N], F32, name="s0acc")
    for w in range(N_WARM):
        nc.tensor.matmul(s0acc[:], ones_sb[:, 0:128], ones_sb[:],
                         start=True, stop=True)

    # ---- transposed fp16 operands ----------------------------------------
    cT = [consts.tile([128, L], F16, tag=f"cT{k}", name=f"cT{k}")
          for k in range(NK)]
    qT = [consts.tile([128, L], F16, tag=f"qT{k}", name=f"qT{k}")
          for k in range(NK)]
    for k in range(NK):
        ksl = slice(k * 128, (k + 1) * 128)
        nc.sync.dma_start(cT[k][:], ct_d[ksl, :])
    for k in range(NK):
        ksl = slice(k * 128, (k + 1) * 128)
        nc.scalar.dma_start(qT[k][:], qt_d[ksl, :])
        # qmodT = qT * cq_weight (per-partition scalar after transpose)
        nc.vector.tensor_scalar_mul(qT[k][:], qT[k][:], cqw32[:, k:k + 1])

    # ---- S1B: s1[j] + bias broadcast across all 128 partitions -----------
    qw_bc = [consts.tile([128, 128], F16, tag=f"qwbc{k}", name=f"qwbc{k}")
             for k in range(NK)]
    for k in range(NK):
        nc.vector.tensor_scalar_mul(qw_bc[k][:], ones_sb[:, 0:128],
                                    qw32[:, k:k + 1])
    bias_row = consts.tile([1, MAIN_N], F16, name="bias_row")
    nc.scalar.activation(bias_row[0:1, :], ones_sb[0:1, :], Copy,
                         scale=bias_sb)

    s1b = consts.tile([128, L], F16, name="s1b")
    s1b_ps = [psum.tile([128, BIG_N], F32, tag="main", name=f"s1b_ps{t}")
              for t in range(2)]
    for k in range(NK):
        for jj in range(4):
            jsl = slice((jj % 2) * MAIN_N, (jj % 2 + 1) * MAIN_N)
            nc.tensor.matmul(s1b_ps[jj // 2][:, jsl], qw_bc[k][:],
                             cT[k][:, jj * MAIN_N:(jj + 1) * MAIN_N],
                             start=(k == 0), stop=False)
    for jj in range(4):
        jsl = slice((jj % 2) * MAIN_N, (jj % 2 + 1) * MAIN_N)
        nc.tensor.matmul(s1b_ps[jj // 2][:, jsl], ones_sb[0:1, 0:128],
                         bias_row[0:1, :], start=False, stop=True)
    nc.vector.tensor_copy(s1b[:, 0:BIG_N], s1b_ps[0][:])
    nc.scalar.activation(s1b[:, BIG_N:L], s1b_ps[1][:], Copy)

    s0col = consts.tile([128, NI], F32, name="s0col")

    # ---- main loop: 16 row chunks ----------------------------------------
    for i in range(NI):
        isl = slice(i * 128, (i + 1) * 128)
        out_sb = outp.tile([128, L], F16, tag="out", name="out_sb")
        pa = psum.tile([128, BIG_N], F32, tag="main", name="pa")
        pb = psum.tile([128, BIG_N], F32, tag="main", name="pb")
        halves = [pa[:, 0:MAIN_N], pa[:, MAIN_N:BIG_N],
                  pb[:, 0:MAIN_N], pb[:, MAIN_N:BIG_N]]
        for k in range(NK):
            for jj in range(4):
                nc.tensor.matmul(halves[jj], cT[k][:, isl],
                                 qT[k][:, jj * MAIN_N:(jj + 1) * MAIN_N],
                                 start=(k == 0), stop=(k == NK - 1))
            nc.tensor.matmul(s0acc[:, i:i + 1], cT[k][:, isl],
                             cw16[:, k:k + 1],
                             start=(k == 0), stop=(k == NK - 1))
        # bounce s0 column through SBUF for the evacuation ops
        nc.vector.tensor_copy(s0col[:, i:i + 1], s0acc[:, i:i + 1])

        # a-tile: fused three-term evacuation on DVE
        nc.vector.scalar_tensor_tensor(out_sb[:, 0:BIG_N], pa[:],
                                       s0col[:, i:i + 1], s1b[:, 0:BIG_N],
                                       ADD, ADD)
        # b-tile: ScalarE adds s0, partner engine adds S1B at fp16 rate
        tmp = tmpp.tile([128, BIG_N], F16, tag="tmp", name="tmp")
        nc.scalar.activation(tmp[:], pb[:], Ident, bias=s0col[:, i:i + 1])
        if i in GPS_CHUNKS:
            nc.gpsimd.tensor_tensor(out_sb[:, BIG_N:L], tmp[:],
                                    s1b[:, BIG_N:L], ADD)
        else:
            nc.vector.tensor_tensor(out_sb[:, BIG_N:L], tmp[:],
                                    s1b[:, BIG_N:L], ADD)
        nc.sync.dma_start(s_d[isl, :], out_sb[:])


def build_nc():
    nc = bacc.Bacc("TRN2", target_bir_lowering=False, debug=False)
    aps = {
        "ct": nc.dram_tensor("ct", [D, L], F16, kind="ExternalInput").ap(),
        "qt": nc.dram_tensor("qt", [D, L], F16, kind="ExternalInput").ap(),
        "w16": nc.dram_tensor("w16", [128, NK], F16, kind="ExternalInput").ap(),
        "w32": nc.dram_tensor("w32", [128, 5], F32, kind="ExternalInput").ap(),
        "s": nc.dram_tensor("s", [L, L], F16, kind="ExternalOutput").ap(),
    }
    with tile.TileContext(nc) as tc:
        with ExitStack() as ctx:
            build_body(ctx, tc, aps)
    nc.compile()
    return nc


def get_nc():
    global _NC_CACHE
    if _NC_CACHE is None:
        _NC_CACHE = build_nc()
    return _NC_CACHE


def kernel(c, q, c_weight, q_weight, cq_weight, bias):
    global LAST_RESULTS
    nc = get_nc()
    c = np.asarray(c, dtype=np.float32)
    q = np.asarray(q, dtype=np.float32)
    cw = np.asarray(c_weight, dtype=np.float32).reshape(D)
    qw = np.asarray(q_weight, dtype=np.float32).reshape(D)
    cqw = np.asarray(cq_weight, dtype=np.float32).reshape(D)
    bias = np.asarray(bias, dtype=np.float32).reshape(1)

    # packed per-partition weights: row p of w16 = [cw[p], cw[128+p]] fp16;
    # row p of w32 = [cqw[p], cqw[128+p], qw[p], qw[128+p], bias]
    w16 = np.ascontiguousarray(cw.reshape(NK, 128).T).astype(np.float16)
    w32 = np.empty((128, 5), dtype=np.float32)
    w32[:, 0:NK] = cqw.reshape(NK, 128).T
    w32[:, NK:2 * NK] = qw.reshape(NK, 128).T
    w32[:, 4] = bias[0]

    in_maps = [
        {
            "ct": np.ascontiguousarray(c[b].T).astype(np.float16),
            "qt": np.ascontiguousarray(q[b].T).astype(np.float16),
            "w16": w16,
            "w32": w32,
        }
        for b in range(B)
    ]
    res = run_bass_kernel_spmd(nc, in_maps, core_ids=list(range(B)), trace=TRACE)
    LAST_RESULTS = res
    return np.stack([res.results[b]["s"].astype(np.float32) for b in range(B)],
                    axis=0)


# revision 11
# speedup vs baseline: 1.3723x; 1.0044x over previous
"""BiAttention similarity kernel for Trainium2, 8-core data-parallel over batch.

Computes, per batch b:
    s0 = c @ c_weight                  # [L, 1]
    s1 = (c @ q_weight)^T              # [1, L]
    s2 = (c * cq_weight) @ q^T         # [L, L]
    s  = s0 + s1 + s2 + bias           # [L, L]

Shapes (hardcoded): B=8, L=2048, D=256, fp32 in/out (device math fp16/fp32,
device output fp16, upcast to fp32 on host).

Distribution: data-parallel over batch, one batch per core. Host hands each
core its shard d-major (transposed) fp16 plus pre-packed per-partition weight
tiles; device output is fp16 (halves the dominant HBM write).

Device plan per core:
  - warmup matmuls on constant data during the input-load window so the PE's
    HAM clock gate reaches 2.4 GHz before real work arrives
  - S1B [128, L]: s1[j]+bias broadcast across partitions, computed on the PE
    as qw_bcast^T @ cT (+ ones^T @ bias_row), evacuated to SBUF fp16
  - s0 column accumulated in one PSUM bank via N=1 matmuls that reuse the
    main loop's stationary cT chunks, bounced [128,1] per chunk to SBUF
  - main GEMM: 16 row chunks; PSUM tiles are [128,1024] fp32 (two banks) so
    each evacuation instruction covers two banks' worth of output
  - evacuation fuses the rank-1 adds, split across three engines:
      a tiles: DVE scalar_tensor_tensor  out = (psum + s0col) + S1B
      b tiles: ScalarE activation (psum + s0col -> fp16 tmp), then
               DVE tensor_tensor (2x fp16 mode) or GpSimd adds S1B
  - output: one 512 KiB DMA per row chunk on the Sync HWDGE ring
"""

import numpy as np
from contextlib import ExitStack

import concourse.bass as bass
import concourse.tile as tile
from concourse import bacc, mybir
from concourse.bass_utils import run_bass_kernel_spmd

F32 = mybir.dt.float32
F16 = mybir.dt.float16

B = 8
L = 2048
D = 256
NK = D // 128          # 2 contraction chunks of 128
NI = L // 128          # 16 row chunks
MAIN_N = 512           # one matmul output <= one PSUM bank
BIG_N = 1024           # evacuation tile: two PSUM banks
N_WARM = 12            # PE warmup matmuls during input load
GPS_CHUNKS = {1, 3, 5, 7, 9, 11, 13, 14}   # chunks whose b-tile partner is GpSimd

TRACE = False
LAST_RESULTS = None

_NC_CACHE = None


def build_body(ctx: ExitStack, tc: tile.TileContext, aps: dict):
    nc = tc.nc
    ct_d, qt_d, w16_d, w32_d, s_d = (
        aps["ct"], aps["qt"], aps["w16"], aps["w32"], aps["s"],
    )
    Copy = mybir.ActivationFunctionType.Copy
    Ident = mybir.ActivationFunctionType.Identity
    ADD = mybir.AluOpType.add

    consts = ctx.enter_context(tc.tile_pool(name="consts", bufs=1))
    psum = ctx.enter_context(tc.tile_pool(name="psum", bufs=3, space="PSUM"))
    psum_s0 = ctx.enter_context(tc.tile_pool(name="psum_s0", bufs=1,
                                             space="PSUM"))
    outp = ctx.enter_context(tc.tile_pool(name="outp", bufs=3))
    tmpp = ctx.enter_context(tc.tile_pool(name="tmpp", bufs=3))

    # ---- packed constants: one fast HWDGE load each -----------------------
    # w16[p] = [cw[p], cw[128+p]] fp16; w32[p] = [cqw k0, cqw k1, qw k0,
    # qw k1, bias] fp32 (bias only meaningful on partition 0)
    w16 = consts.tile([128, NK], F16, name="w16")
    nc.sync.dma_start(w16[:], w16_d[:, :])
    w32 = consts.tile([128, 5], F32, name="w32")
    nc.sync.dma_start(w32[:], w32_d[:, :])
    cw16 = w16
    cqw32 = w32[:, 0:NK]
    qw32 = w32[:, NK:2 * NK]
    bias_sb = w32[0:1, 4:5]

    ones_sb = consts.tile([128, MAIN_N], F16, name="ones_sb")
    nc.vector.memset(ones_sb[:], 1.0)

    # ---- PE warmup: release the HAM clock gate during the load window ----
    # The s0acc banks double as the warmup target; junk lands in columns the
    # s0 matmuls never touch (and start=True clears has_written anyway).
    # Two s0acc banks ping-pong across chunks so the PE's s0 matmul of chunk
    # i only serializes against the DVE bounce of chunk i-2 (2 chunks slack).
    s0acc = [psum_s0.tile([128, MAIN_N], F32, tag=f"s0acc{t}",
                          name=f"s0acc{t}") for t in range(2)]
    for w in range(N_WARM):
        nc.tensor.matmul(s0acc[w % 2][:], ones_sb[:, 0:128], ones_sb[:],
                         start=True, stop=True)

    # ---- transposed fp16 operands ----------------------------------------
    cT = [consts.tile([128, L], F16, tag=f"cT{k}", name=f"cT{k}")
          for k in range(NK)]
    qT = [consts.tile([128, L], F16, tag=f"qT{k}", name=f"qT{k}")
          for k in range(NK)]
    for k in range(NK):
        ksl = slice(k * 128, (k + 1) * 128)
        nc.sync.dma_start(cT[k][:], ct_d[ksl, :])
    for k in range(NK):
        ksl = slice(k * 128, (k + 1) * 128)
        nc.scalar.dma_start(qT[k][:], qt_d[ksl, :])
        # qmodT = qT * cq_weight (per-partition scalar after transpose)
        nc.vector.tensor_scalar_mul(qT[k][:], qT[k][:], cqw32[:, k:k + 1])

    # ---- S1B: s1[j] + bias broadcast across all 128 partitions -----------
    qw_bc = [consts.tile([128, 128], F16, tag=f"qwbc{k}", name=f"qwbc{k}")
             for k in range(NK)]
    for k in range(NK):
        nc.vector.tensor_scalar_mul(qw_bc[k][:], ones_sb[:, 0:128],
                                    qw32[:, k:k + 1])
    bias_row = consts.tile([1, MAIN_N], F16, name="bias_row")
    nc.scalar.activation(bias_row[0:1, :], ones_sb[0:1, :], Copy,
                         scale=bias_sb)

    s1b = consts.tile([128, L], F16, name="s1b")
    s1b_ps = [psum.tile([128, BIG_N], F32, tag="main", name=f"s1b_ps{t}")
              for t in range(2)]
    for k in range(NK):
        for jj in range(4):
            jsl = slice((jj % 2) * MAIN_N, (jj % 2 + 1) * MAIN_N)
            nc.tensor.matmul(s1b_ps[jj // 2][:, jsl], qw_bc[k][:],
                             cT[k][:, jj * MAIN_N:(jj + 1) * MAIN_N],
                             start=(k == 0), stop=False)
    for jj in range(4):
        jsl = slice((jj % 2) * MAIN_N, (jj % 2 + 1) * MAIN_N)
        nc.tensor.matmul(s1b_ps[jj // 2][:, jsl], ones_sb[0:1, 0:128],
                         bias_row[0:1, :], start=False, stop=True)
    nc.vector.tensor_copy(s1b[:, 0:BIG_N], s1b_ps[0][:])
    nc.scalar.activation(s1b[:, BIG_N:L], s1b_ps[1][:], Copy)

    s0col = consts.tile([128, NI], F32, name="s0col")

    # ---- main loop: 16 row chunks ----------------------------------------
    for i in range(NI):
        isl = slice(i * 128, (i + 1) * 128)
        out_sb = outp.tile([128, L], F16, tag="out", name="out_sb")
        pa = psum.tile([128, BIG_N], F32, tag="main", name="pa")
        pb = psum.tile([128, BIG_N], F32, tag="main", name="pb")
        halves = [pa[:, 0:MAIN_N], pa[:, MAIN_N:BIG_N],
                  pb[:, 0:MAIN_N], pb[:, MAIN_N:BIG_N]]
        for k in range(NK):
            for jj in range(4):
                nc.tensor.matmul(halves[jj], cT[k][:, isl],
                                 qT[k][:, jj * MAIN_N:(jj + 1) * MAIN_N],
                                 start=(k == 0), stop=(k == NK - 1))
            nc.tensor.matmul(s0acc[i % 2][:, i // 2:i // 2 + 1],
                             cT[k][:, isl], cw16[:, k:k + 1],
                             start=(k == 0), stop=(k == NK - 1))
        # bounce s0 column through SBUF for the evacuation ops
        nc.vector.tensor_copy(s0col[:, i:i + 1],
                              s0acc[i % 2][:, i // 2:i // 2 + 1])

        # a-tile: fused three-term evacuation on DVE
        nc.vector.scalar_tensor_tensor(out_sb[:, 0:BIG_N], pa[:],
                                       s0col[:, i:i + 1], s1b[:, 0:BIG_N],
                                       ADD, ADD)
        # b-tile: ScalarE adds s0, partner engine adds S1B at fp16 rate
        tmp = tmpp.tile([128, BIG_N], F16, tag="tmp", name="tmp")
        nc.scalar.activation(tmp[:], pb[:], Ident, bias=s0col[:, i:i + 1])
        if i in GPS_CHUNKS:
            nc.gpsimd.tensor_tensor(out_sb[:, BIG_N:L], tmp[:],
                                    s1b[:, BIG_N:L], ADD)
        else:
            nc.vector.tensor_tensor(out_sb[:, BIG_N:L], tmp[:],
                                    s1b[:, BIG_N:L], ADD)
        nc.sync.dma_start(s_d[isl, :], out_sb[:])


def build_nc():
    nc = bacc.Bacc("TRN2", target_bir_lowering=False, debug=False)
    aps = {
        "ct": nc.dram_tensor("ct", [D, L], F16, kind="ExternalInput").ap(),
        "qt": nc.dram_tensor("qt", [D, L], F16, kind="ExternalInput").ap(),
        "w16": nc.dram_tensor("w16", [128, NK], F16, kind="ExternalInput").ap(),
        "w32": nc.dram_tensor("w32", [128, 5], F32, kind="ExternalInput").ap(),
        "s": nc.dram_tensor("s", [L, L], F16, kind="ExternalOutput").ap(),
    }
    with tile.TileContext(nc) as tc:
        with ExitStack() as ctx:
            build_body(ctx, tc, aps)
    nc.compile()
    return nc


def get_nc():
    global _NC_CACHE
    if _NC_CACHE is None:
        _NC_CACHE = build_nc()
    return _NC_CACHE


def kernel(c, q, c_weight, q_weight, cq_weight, bias):
    global LAST_RESULTS
    nc = get_nc()
    c = np.asarray(c, dtype=np.float32)
    q = np.asarray(q, dtype=np.float32)
    cw = np.asarray(c_weight, dtype=np.float32).reshape(D)
    qw = np.asarray(q_weight, dtype=np.float32).reshape(D)
    cqw = np.asarray(cq_weight, dtype=np.float32).reshape(D)
    bias = np.asarray(bias, dtype=np.float32).reshape(1)

    # packed per-partition weights: row p of w16 = [cw[p], cw[128+p]] fp16;
    # row p of w32 = [cqw[p], cqw[128+p], qw[p], qw[128+p], bias]
    w16 = np.ascontiguousarray(cw.reshape(NK, 128).T).astype(np.float16)
    w32 = np.empty((128, 5), dtype=np.float32)
    w32[:, 0:NK] = cqw.reshape(NK, 128).T
    w32[:, NK:2 * NK] = qw.reshape(NK, 128).T
    w32[:, 4] = bias[0]

    in_maps = [
        {
            "ct": np.ascontiguousarray(c[b].T).astype(np.float16),
            "qt": np.ascontiguousarray(q[b].T).astype(np.float16),
            "w16": w16,
            "w32": w32,
        }
        for b in range(B)
    ]
    res = run_bass_kernel_spmd(nc, in_maps, core_ids=list(range(B)), trace=TRACE)
    LAST_RESULTS = res
    return np.stack([res.results[b]["s"].astype(np.float32) for b in range(B)],
                    axis=0)
